# revision 3
# baseline (speedup 1.0000x reference)
"""Trainium2 Bass kernel for GQA multi-head attention (B=2, S=2048, H=2048,
32 q heads / 8 kv heads / head_dim 64, RoPE, causal softmax, output proj).

Sharding over 8 NeuronCores: core c handles batch b=c//4 and kv-head pair
j=c%4 (kv heads 2j, 2j+1 -> q heads 8j..8j+7).  Each core computes its
q/k/v projections from a replicated (per-batch) x^T, runs causal attention
for its 8 q heads in a transposed-scores layout (lazy softmax via a
[v|ones] matmul column), and produces a partial output-projection which the
host sums across the 4 cores of each batch.

All matmuls run in fp32r (full fp32 storage, reduced-precision PE mode,
4x the fp32 matmul rate).
"""

import numpy as np

B, S, H = 2, 2048, 2048
NH, NKV, HD = 32, 8, 64
P = 128
ST = 512           # sequence tile (free dim of most matmuls)
NT = S // ST       # 4 sequence tiles
KC = H // P        # 16 contraction chunks for projections
NCORES = 8

_CACHE = {}


def _build(reps=1):
    import concourse.bass as bass
    import concourse.mybir as mybir
    from concourse import bacc
    from concourse.tile import TileContext

    f32 = mybir.dt.float32
    f32r = mybir.dt.float32r
    AF = mybir.ActivationFunctionType
    OP = mybir.AluOpType

    nc = bacc.Bacc("TRN2", target_bir_lowering=False, debug=False,
                   num_devices=NCORES)

    xT_d = nc.dram_tensor("xT", [H, S], f32, kind="ExternalInput")
    wq_d = nc.dram_tensor("wq", [H, 512], f32, kind="ExternalInput")
    wk_d = nc.dram_tensor("wk", [H, 128], f32, kind="ExternalInput")
    wv_d = nc.dram_tensor("wv", [H, 128], f32, kind="ExternalInput")
    wo_d = nc.dram_tensor("wo", [512, H], f32, kind="ExternalInput")
    c2_d = nc.dram_tensor("c2", [P, S], f32, kind="ExternalInput")
    s2_d = nc.dram_tensor("s2", [P, S], f32, kind="ExternalInput")
    tri_d = nc.dram_tensor("tri", [P, P], f32, kind="ExternalInput")
    out_d = nc.dram_tensor("out", [S, H], f32, kind="ExternalOutput")

    with TileContext(nc) as tc:
        # ---- pools that live for the whole kernel
        with tc.tile_pool(name="const", bufs=1) as constp, \
             tc.tile_pool(name="qkv", bufs=1) as qkvp, \
             tc.tile_pool(name="attn", bufs=1) as attnp:

            c2 = constp.tile([P, S], f32)
            s2 = constp.tile([P, S], f32)
            tri = constp.tile([P, P], f32)
            nc.sync.dma_start(c2[:], c2_d.ap())
            nc.sync.dma_start(s2[:], s2_d.ap())
            nc.sync.dma_start(tri[:], tri_d.ap())
            ones_f = constp.tile([P, 1], f32)
            nc.vector.memset(ones_f[:], 1.0)
            ones_row = constp.tile([1, 64], f32r)
            nc.vector.tensor_copy(ones_row[:], ones_f[0:1, :].to_broadcast((1, 64)))
            ident = constp.tile([64, 64], f32)
            from concourse.masks import make_identity
            make_identity(nc, ident[:])

            qT = qkvp.tile([P, 4, S], f32r)        # 4 head pairs
            kTd = [qkvp.tile([P, S], f32r, tag=f"ktd{kv}", name=f"ktd{kv}") for kv in range(2)]
            vv = [qkvp.tile([P, KC, 65], f32r, tag=f"v{kv}", name=f"v{kv}") for kv in range(2)]
            attnT = attnp.tile([P, 4, S], f32r)

            # ones column of [v | 1] tiles
            for kv in range(2):
                nc.vector.tensor_copy(
                    vv[kv][:, :, 64:65],
                    ones_f[:, None, 0:1].to_broadcast((P, KC, 1)))

            for rep in range(reps):
                # ============ Phase A: QKV projection + RoPE ============
                with tc.tile_pool(name="wpool", bufs=1) as wp, \
                     tc.tile_pool(name="xpool", bufs=4) as xp, \
                     tc.tile_pool(name="ropet", bufs=2) as rp, \
                     tc.tile_pool(name="psA", bufs=6, space="PSUM") as psA, \
                     tc.tile_pool(name="psT", bufs=2, space="PSUM") as psT:

                    wq = wp.tile([P, KC, 512], f32r)
                    wk = wp.tile([P, KC, 128], f32r)
                    wv = wp.tile([P, KC, 128], f32r)
                    nc.sync.dma_start(
                        wq[:], wq_d.ap().rearrange("(ko p) m -> p ko m", p=P).bitcast(f32r))
                    nc.sync.dma_start(
                        wk[:], wk_d.ap().rearrange("(ko p) m -> p ko m", p=P).bitcast(f32r))
                    nc.sync.dma_start(
                        wv[:], wv_d.ap().rearrange("(ko p) m -> p ko m", p=P).bitcast(f32r))

                    for T in range(NT):
                        ts = slice(ST * T, ST * (T + 1))
                        xk = [xp.tile([P, ST], f32r, tag="xk", name=f"xk{_}") for _ in range(KC)]
                        for k in range(KC):
                            nc.sync.dma_start(
                                xk[k][:],
                                xT_d.ap()[P * k:P * (k + 1), ts].bitcast(f32r))
                        ps = [psA.tile([P, ST], f32, tag="proj", name=f"proj{_}") for _ in range(6)]
                        for k in range(KC):
                            st, sp = (k == 0), (k == KC - 1)
                            for m in range(4):
                                nc.tensor.matmul(ps[m][:], wq[:, k, 128 * m:128 * (m + 1)],
                                                 xk[k][:], start=st, stop=sp)
                            nc.tensor.matmul(ps[4][:], wk[:, k], xk[k][:], start=st, stop=sp)
                            nc.tensor.matmul(ps[5][:], wv[:, k], xk[k][:], start=st, stop=sp)

                        # rope epilogue for q chunks (into qT) and k chunk
                        def rope(psrc, dst):
                            raw = rp.tile([P, ST], f32, tag="raw")
                            nc.vector.tensor_copy(raw[:], psrc[:])
                            swp = rp.tile([P, ST], f32, tag="swp")
                            for g in range(2):
                                b0 = 64 * g
                                nc.sync.dma_start(swp[b0:b0 + 32], raw[b0 + 32:b0 + 64])
                                nc.sync.dma_start(swp[b0 + 32:b0 + 64], raw[b0:b0 + 32])
                            t2 = rp.tile([P, ST], f32, tag="t2")
                            nc.vector.tensor_tensor(t2[:], raw[:], c2[:, ts], OP.mult)
                            t3 = rp.tile([P, ST], f32, tag="t3")
                            nc.vector.tensor_tensor(t3[:], swp[:], s2[:, ts], OP.mult)
                            nc.vector.tensor_tensor(dst, t2[:], t3[:], OP.add)

                        for m in range(4):
                            rope(ps[m], qT[:, m, ts])
                        ktmp = rp.tile([P, ST], f32r, tag="ktmp")
                        rope(ps[4], ktmp[:])
                        for kv in range(2):
                            nc.sync.dma_start(kTd[kv][0:64, ts], ktmp[64 * kv:64 * kv + 64])
                            nc.sync.dma_start(kTd[kv][64:128, ts], ktmp[64 * kv:64 * kv + 64])

                        # v: transpose vT chunk into [s, d] layout
                        vraw = rp.tile([P, ST], f32, tag="vraw")
                        nc.vector.tensor_copy(vraw[:], ps[5][:])
                        vraw2 = rp.tile([64, ST], f32, tag="vraw2")
                        nc.sync.dma_start(vraw2[:], vraw[64:128])
                        for blk in range(4):
                            bs = slice(128 * blk, 128 * (blk + 1))
                            for kv, vsrc in ((0, vraw), (1, vraw2)):
                                pst = psT.tile([P, 64], f32, tag="pst")
                                nc.tensor.transpose(pst[:], vsrc[0:64, bs], ident[:])
                                nc.vector.tensor_copy(vv[kv][:, 4 * T + blk, 0:64], pst[:])

                # ============ Phase B: causal attention ============
                with tc.tile_pool(name="expp", bufs=6) as ep, \
                     tc.tile_pool(name="nrm", bufs=2) as np_, \
                     tc.tile_pool(name="psS", bufs=4, space="PSUM") as psS, \
                     tc.tile_pool(name="psV", bufs=2, space="PSUM") as psV, \
                     tc.tile_pool(name="psB", bufs=2, space="PSUM") as psB:

                    for pair in range(4):
                        kv = pair // 2
                        for t in range(NT):
                            ts = slice(ST * t, ST * (t + 1))
                            av = [psV.tile([65, ST], f32, tag="av", name=f"av{_}") for _ in range(2)]
                            nch = 4 * (t + 1)
                            for k in range(nch):
                                c0 = 128 * (k - 4 * t) if k >= 4 * t else 0
                                kk = slice(P * k, P * (k + 1))
                                for par in range(2):
                                    p0 = 64 * par
                                    pss = psS.tile([P, ST], f32, tag="sc")
                                    nc.tensor.matmul(
                                        pss[:, c0:],
                                        kTd[kv][p0:p0 + 64, kk],
                                        qT[p0:p0 + 64, pair, ST * t + c0:ST * (t + 1)])
                                    if k >= 4 * t:
                                        nc.vector.tensor_tensor(
                                            pss[:, c0:c0 + 128], pss[:, c0:c0 + 128],
                                            tri[:], OP.add)
                                    ex = ep.tile([P, ST], f32r, tag="expS")
                                    nc.scalar.activation(ex[:, c0:], pss[:, c0:],
                                                         AF.Exp, scale=0.125)
                                    nc.tensor.matmul(av[par][:, c0:], vv[kv][:, k],
                                                     ex[:, c0:],
                                                     start=(k == 0), stop=(k == nch - 1))
                            # normalize both heads of the pair
                            for par in range(2):
                                sbav = np_.tile([65, ST], f32, tag="sbav")
                                nc.vector.tensor_copy(sbav[:], av[par][:])
                                den = np_.tile([1, ST], f32, tag="den")
                                nc.sync.dma_start(den[:], sbav[64:65, :])
                                rcp = np_.tile([1, ST], f32r, tag="rcp")
                                with nc.allow_low_precision(reason="softmax denom f32r"):
                                    nc.vector.reciprocal(rcp[:], den[:])
                                psb = psB.tile([64, ST], f32, tag="bc")
                                nc.tensor.matmul(psb[:], ones_row[:], rcp[:])
                                if par == 0:
                                    nc.vector.tensor_tensor(
                                        attnT[0:64, pair, ts], sbav[0:64, :], psb[:],
                                        OP.mult)
                                else:
                                    otmp = np_.tile([64, ST], f32r, tag="otmp")
                                    nc.vector.tensor_tensor(otmp[:], sbav[0:64, :],
                                                            psb[:], OP.mult)
                                    nc.sync.dma_start(attnT[64:128, pair, ts], otmp[:])

                # ============ Phase C: output projection (partial) ============
                with tc.tile_pool(name="wop", bufs=2) as wop, \
                     tc.tile_pool(name="outp", bufs=3) as outp, \
                     tc.tile_pool(name="psC", bufs=2, space="PSUM") as psC:
                    wo_view = wo_d.ap().rearrange("(cp p) e -> p cp e", p=P)
                    for e in range(4):
                        es = slice(ST * e, ST * (e + 1))
                        wo_t = wop.tile([P, 4, ST], f32r, tag="wo")
                        nc.sync.dma_start(wo_t[:], wo_view[:, :, es].bitcast(f32r))
                        for sb in range(S // P):
                            pso = psC.tile([P, ST], f32, tag="op")
                            for cp in range(4):
                                nc.tensor.matmul(pso[:],
                                                 attnT[:, cp, P * sb:P * (sb + 1)],
                                                 wo_t[:, cp, :],
                                                 start=(cp == 0), stop=(cp == 3))
                            ot = outp.tile([P, ST], f32, tag="ot")
                            nc.vector.tensor_copy(ot[:], pso[:])
                            nc.sync.dma_start(out_d.ap()[P * sb:P * (sb + 1), es], ot[:])

    nc.compile()
    return nc


def _host_prep(x, rotary_cos, rotary_sin, Wq, Wk, Wv, Wo):
    x = np.asarray(x, np.float32)
    cos = np.asarray(rotary_cos, np.float32)
    sin = np.asarray(rotary_sin, np.float32)
    Wq = np.asarray(Wq, np.float32)
    Wk = np.asarray(Wk, np.float32)
    Wv = np.asarray(Wv, np.float32)
    Wo = np.asarray(Wo, np.float32)

    c2 = np.empty((P, S), np.float32)
    s2 = np.empty((P, S), np.float32)
    for p in range(P):
        c2[p] = cos[:, p % 32]
        s2[p] = sin[:, p % 32] * (-1.0 if (p % 64) < 32 else 1.0)
    tri = np.where(np.arange(P)[:, None] > np.arange(P)[None, :],
                   np.float32(-1e30), np.float32(0.0)).astype(np.float32)

    xTs = [np.ascontiguousarray(x[b].T) for b in range(B)]
    in_maps = []
    for c in range(NCORES):
        b, j = divmod(c, 4)
        in_maps.append({
            "xT": xTs[b],
            "wq": np.ascontiguousarray(Wq[:, 512 * j:512 * (j + 1)]),
            "wk": np.ascontiguousarray(Wk[:, 128 * j:128 * (j + 1)]),
            "wv": np.ascontiguousarray(Wv[:, 128 * j:128 * (j + 1)]),
            "wo": np.ascontiguousarray(Wo[512 * j:512 * (j + 1), :]),
            "c2": c2, "s2": s2, "tri": tri,
        })
    return in_maps


def kernel(x, rotary_cos, rotary_sin, Wq, Wk, Wv, Wo, reps=1, _want_res=False):
    from concourse.bass_utils import run_bass_kernel_spmd
    if reps not in _CACHE:
        _CACHE[reps] = _build(reps)
    nc = _CACHE[reps]
    in_maps = _host_prep(x, rotary_cos, rotary_sin, Wq, Wk, Wv, Wo)
    res = run_bass_kernel_spmd(nc, in_maps, list(range(NCORES)))
    out = np.empty((B, S, H), np.float32)
    for b in range(B):
        acc = res.results[4 * b]["out"].astype(np.float64)
        for j in range(1, 4):
            acc += res.results[4 * b + j]["out"]
        out[b] = acc.astype(np.float32)
    if _want_res:
        return out, res
    return out


# revision 12
# speedup vs baseline: 8779.8089x; 8779.8089x over previous
"""Trainium2 Bass kernel for GQA multi-head attention (B=2, S=2048, H=2048,
32 q heads / 8 kv heads / head_dim 64, RoPE, causal softmax, output proj).

Sharding over 8 NeuronCores: core c handles batch b=c//4 and kv-head pair
j=c%4 (kv heads 2j, 2j+1 -> q heads 8j..8j+7).  Each core computes its
q/k/v projections from a replicated (per-batch) x^T, runs causal attention
for its 8 q heads in a transposed-scores layout (lazy softmax via a
[v|ones] matmul column), and produces a partial output-projection which the
host sums across the 4 cores of each batch.

All matmuls run in fp32r (full fp32 storage, reduced-precision PE mode,
4x the fp32 matmul rate).
"""

import numpy as np

B, S, H = 2, 2048, 2048
NH, NKV, HD = 32, 8, 64
P = 128
ST = 512           # sequence tile (free dim of most matmuls)
NT = S // ST       # 4 sequence tiles
KC = H // P        # 16 contraction chunks for projections
NCORES = 8

_CACHE = {}


def _build(reps=1, phases='ABC'):
    import concourse.bass as bass
    import concourse.mybir as mybir
    from concourse import bacc
    from concourse.tile import TileContext

    f32 = mybir.dt.float32
    f32r = mybir.dt.float32r
    AF = mybir.ActivationFunctionType
    OP = mybir.AluOpType

    nc = bacc.Bacc("TRN2", target_bir_lowering=False, debug=False,
                   num_devices=NCORES)

    xT_d = nc.dram_tensor("xT", [H, S], f32, kind="ExternalInput")
    wq_d = nc.dram_tensor("wq", [H, 512], f32, kind="ExternalInput")
    wk_d = nc.dram_tensor("wk", [H, 128], f32, kind="ExternalInput")
    wv_d = nc.dram_tensor("wv", [H, 128], f32, kind="ExternalInput")
    wo_d = nc.dram_tensor("wo", [512, H], f32, kind="ExternalInput")
    c2_d = nc.dram_tensor("c2", [P, S], f32, kind="ExternalInput")
    s2_d = nc.dram_tensor("s2", [P, S], f32, kind="ExternalInput")
    tri_d = nc.dram_tensor("tri", [P, P], f32, kind="ExternalInput")
    out_d = nc.dram_tensor("out", [S, H], f32, kind="ExternalOutput")

    with TileContext(nc) as tc:
        # ---- pools that live for the whole kernel
        with tc.tile_pool(name="const", bufs=1) as constp, \
             tc.tile_pool(name="qkv", bufs=1) as qkvp, \
             tc.tile_pool(name="attn", bufs=1) as attnp:

            c2 = constp.tile([P, S], f32)
            s2 = constp.tile([P, S], f32)
            tri = constp.tile([P, P], f32)
            ones_f = constp.tile([P, 1], f32)
            nc.vector.memset(ones_f[:], 1.0)
            ones_row = constp.tile([1, 64], f32r)
            nc.vector.tensor_copy(ones_row[:], ones_f[0:1, :].to_broadcast((1, 64)))
            ident = constp.tile([64, 64], f32)
            from concourse.masks import make_identity
            make_identity(nc, ident[:])

            qT = qkvp.tile([P, 4, S], f32r)        # 4 head pairs
            kTd = [qkvp.tile([P, S], f32r, tag=f"ktd{kv}", name=f"ktd{kv}") for kv in range(2)]
            vv = [qkvp.tile([P, KC, 65], f32r, tag=f"v{kv}", name=f"v{kv}") for kv in range(2)]
            attnT = attnp.tile([P, 4, S], f32r)

            # ones column of [v | 1] tiles
            for kv in range(2):
                nc.vector.tensor_copy(
                    vv[kv][:, :, 64:65],
                    ones_f[:, None, 0:1].to_broadcast((P, KC, 1)))

            for rep in range(reps):
                # ============ Phase A: QKV projection + RoPE ============
                if 'A' not in phases:
                    break
                with tc.tile_pool(name="wpool", bufs=1) as wp, \
                     tc.tile_pool(name="xpool", bufs=3) as xp, \
                     tc.tile_pool(name="ropet", bufs=2) as rp, \
                     tc.tile_pool(name="psA", bufs=6, space="PSUM") as psA, \
                     tc.tile_pool(name="psT", bufs=2, space="PSUM") as psT:

                    wq = wp.tile([P, KC, 512], f32r)
                    wk = wp.tile([P, KC, 128], f32r)
                    wv = wp.tile([P, KC, 128], f32r)
                    wq_view = wq_d.ap().rearrange("(ko p) m -> p ko m", p=P).bitcast(f32r)
                    wk_view = wk_d.ap().rearrange("(ko p) m -> p ko m", p=P).bitcast(f32r)
                    wv_view = wv_d.ap().rearrange("(ko p) m -> p ko m", p=P).bitcast(f32r)
                    # critical prefix on SP (k=0..3 slices); remainder on ACT queue
                    nc.sync.dma_start(wq[:, 0:4], wq_view[:, 0:4])
                    nc.sync.dma_start(wk[:, 0:4], wk_view[:, 0:4])
                    nc.sync.dma_start(wv[:, 0:4], wv_view[:, 0:4])
                    nc.scalar.dma_start(wq[:, 4:10], wq_view[:, 4:10])
                    nc.scalar.dma_start(wq[:, 10:16], wq_view[:, 10:16])
                    nc.scalar.dma_start(wk[:, 4:16], wk_view[:, 4:16])
                    nc.scalar.dma_start(wv[:, 4:16], wv_view[:, 4:16])
                    if rep == 0:
                        nc.scalar.dma_start(c2[:], c2_d.ap())
                        nc.scalar.dma_start(s2[:], s2_d.ap())
                        nc.scalar.dma_start(tri[:], tri_d.ap())

                    for T in range(NT):
                        ts = slice(ST * T, ST * (T + 1))
                        ps = [psA.tile([P, ST], f32, tag="proj", name=f"proj{_}") for _ in range(6)]
                        for half in range(2):
                            xg = [xp.tile([P, 4, ST], f32r, tag="xk", name=f"xk{half}_{_}")
                                  for _ in range(2)]
                            for g in range(2):
                                g0 = 2 * half + g
                                nc.sync.dma_start(
                                    xg[g][:],
                                    xT_d.ap()[512 * g0:512 * (g0 + 1), ts]
                                    .rearrange("(kc p) s -> p kc s", p=P).bitcast(f32r))
                            for kk in range(8):
                                k = 8 * half + kk
                                xkc = xg[kk // 4][:, kk % 4]
                                st, sp = (k == 0), (k == KC - 1)
                                for m in range(4):
                                    nc.tensor.matmul(ps[m][:], wq[:, k, 128 * m:128 * (m + 1)],
                                                     xkc, start=st, stop=sp)
                                nc.tensor.matmul(ps[4][:], wk[:, k], xkc, start=st, stop=sp)
                                nc.tensor.matmul(ps[5][:], wv[:, k], xkc, start=st, stop=sp)

                        # rope epilogue for q chunks (into qT) and k chunk
                        def rope(psrc, dst):
                            raw = rp.tile([P, ST], f32, tag="raw")
                            nc.vector.tensor_copy(raw[:], psrc[:])
                            swp = rp.tile([P, ST], f32, tag="swp")
                            for g in range(2):
                                b0 = 64 * g
                                nc.gpsimd.dma_start(swp[b0:b0 + 32], raw[b0 + 32:b0 + 64])
                                nc.gpsimd.dma_start(swp[b0 + 32:b0 + 64], raw[b0:b0 + 32])
                            nc.vector.tensor_tensor(raw[:], raw[:], c2[:, ts], OP.mult)
                            nc.vector.tensor_tensor(swp[:], swp[:], s2[:, ts], OP.mult)
                            nc.vector.tensor_tensor(dst, raw[:], swp[:], OP.add)

                        for m in range(4):
                            rope(ps[m], qT[:, m, ts])
                        ktmp = rp.tile([P, ST], f32r, tag="ktmp")
                        rope(ps[4], ktmp[:])
                        for kv in range(2):
                            nc.gpsimd.dma_start(kTd[kv][0:64, ts], ktmp[64 * kv:64 * kv + 64])
                            nc.gpsimd.dma_start(kTd[kv][64:128, ts], ktmp[64 * kv:64 * kv + 64])

                        # v: transpose vT chunk into [s, d] layout
                        vraw = rp.tile([P, ST], f32, tag="vraw")
                        nc.vector.tensor_copy(vraw[:], ps[5][:])
                        vraw2 = rp.tile([64, ST], f32, tag="vraw2")
                        nc.gpsimd.dma_start(vraw2[:], vraw[64:128])
                        for blk in range(4):
                            bs = slice(128 * blk, 128 * (blk + 1))
                            for kv, vsrc in ((0, vraw), (1, vraw2)):
                                pst = psT.tile([P, 64], f32, tag="pst")
                                nc.tensor.transpose(pst[:], vsrc[0:64, bs], ident[:])
                                nc.vector.tensor_copy(vv[kv][:, 4 * T + blk, 0:64], pst[:])

                # ===== Phase B+C fused: attention (t-major) + output proj =====
                if 'B' not in phases:
                    continue
                with tc.tile_pool(name="expp", bufs=6) as ep, \
                     tc.tile_pool(name="nrm", bufs=2) as np_, \
                     tc.tile_pool(name="wop", bufs=1) as wop, \
                     tc.tile_pool(name="outp", bufs=3) as outp, \
                     tc.tile_pool(name="psS", bufs=2, space="PSUM") as psS, \
                     tc.tile_pool(name="psV", bufs=2, space="PSUM") as psV, \
                     tc.tile_pool(name="psM", bufs=2, space="PSUM") as psM:

                    wo_t = wop.tile([P, 4, H], f32r, tag="wo")
                    nc.scalar.dma_start(
                        wo_t[:],
                        wo_d.ap().rearrange("(cp p) e -> p cp e", p=P).bitcast(f32r))

                    for t in range(NT):
                        ts = slice(ST * t, ST * (t + 1))
                        for pair in range(4):
                            kv = pair // 2
                            av = [psV.tile([65, ST], f32, tag="av", name=f"av{_}")
                                  for _ in range(2)]
                            nch = 4 * (t + 1)
                            for k in range(nch):
                                c0 = 128 * (k - 4 * t) if k >= 4 * t else 0
                                kk = slice(P * k, P * (k + 1))
                                pss = psS.tile([P, 2, ST], f32, tag="sc")
                                for par in range(2):
                                    p0 = 64 * par
                                    nc.tensor.matmul(
                                        pss[:, par, c0:],
                                        kTd[kv][p0:p0 + 64, kk],
                                        qT[p0:p0 + 64, pair, ST * t + c0:ST * (t + 1)])
                                if k >= 4 * t:
                                    nc.vector.tensor_tensor(
                                        pss[:, :, c0:c0 + 128], pss[:, :, c0:c0 + 128],
                                        tri[:, None, :].to_broadcast((P, 2, 128)),
                                        OP.add)
                                ex = ep.tile([P, 2, ST], f32r, tag="expS")
                                nc.scalar.activation(ex[:, :, c0:], pss[:, :, c0:],
                                                     AF.Exp, scale=0.125)
                                for par in range(2):
                                    nc.tensor.matmul(av[par][:, c0:], vv[kv][:, k],
                                                     ex[:, par, c0:],
                                                     start=(k == 0), stop=(k == nch - 1))
                            # normalize both heads of the pair
                            for par in range(2):
                                sbav = np_.tile([65, ST], f32, tag="sbav")
                                nc.vector.tensor_copy(sbav[:], av[par][:])
                                den = np_.tile([1, ST], f32, tag="den")
                                nc.gpsimd.dma_start(den[:], sbav[64:65, :])
                                rcp = np_.tile([1, ST], f32r, tag="rcp")
                                with nc.allow_low_precision(reason="softmax denom f32r"):
                                    nc.vector.reciprocal(rcp[:], den[:])
                                psb = psM.tile([P, ST], f32, tag="misc", name="psb")
                                nc.tensor.matmul(psb[0:64, :], ones_row[:], rcp[:])
                                if par == 0:
                                    nc.vector.tensor_tensor(
                                        attnT[0:64, pair, ts], sbav[0:64, :],
                                        psb[0:64, :], OP.mult)
                                else:
                                    otmp = np_.tile([64, ST], f32r, tag="otmp")
                                    nc.vector.tensor_tensor(otmp[:], sbav[0:64, :],
                                                            psb[0:64, :], OP.mult)
                                    nc.gpsimd.dma_start(attnT[64:128, pair, ts], otmp[:])

                        # output projection for this t's token blocks
                        for sb in (range(4 * t, 4 * (t + 1)) if 'C' in phases else ()):
                            ot = outp.tile([P, 4, ST], f32, tag="ot")
                            for e in range(4):
                                es = slice(ST * e, ST * (e + 1))
                                pso = psM.tile([P, ST], f32, tag="misc", name="pso")
                                for cp in range(4):
                                    nc.tensor.matmul(pso[:],
                                                     attnT[:, cp, P * sb:P * (sb + 1)],
                                                     wo_t[:, cp, es],
                                                     start=(cp == 0), stop=(cp == 3))
                                if e % 2 == 0:
                                    nc.vector.tensor_copy(ot[:, e], pso[:])
                                else:
                                    nc.scalar.copy(ot[:, e], pso[:])
                            nc.sync.dma_start(
                                out_d.ap()[P * sb:P * (sb + 1), :],
                                ot[:].rearrange("p e s -> p (e s)"))

    nc.compile()
    return nc


def _host_prep(x, rotary_cos, rotary_sin, Wq, Wk, Wv, Wo):
    x = np.asarray(x, np.float32)
    cos = np.asarray(rotary_cos, np.float32)
    sin = np.asarray(rotary_sin, np.float32)
    Wq = np.asarray(Wq, np.float32)
    Wk = np.asarray(Wk, np.float32)
    Wv = np.asarray(Wv, np.float32)
    Wo = np.asarray(Wo, np.float32)

    c2 = np.empty((P, S), np.float32)
    s2 = np.empty((P, S), np.float32)
    for p in range(P):
        c2[p] = cos[:, p % 32]
        s2[p] = sin[:, p % 32] * (-1.0 if (p % 64) < 32 else 1.0)
    tri = np.where(np.arange(P)[:, None] > np.arange(P)[None, :],
                   np.float32(-1e30), np.float32(0.0)).astype(np.float32)

    xTs = [np.ascontiguousarray(x[b].T) for b in range(B)]
    in_maps = []
    for c in range(NCORES):
        b, j = divmod(c, 4)
        in_maps.append({
            "xT": xTs[b],
            "wq": np.ascontiguousarray(Wq[:, 512 * j:512 * (j + 1)]),
            "wk": np.ascontiguousarray(Wk[:, 128 * j:128 * (j + 1)]),
            "wv": np.ascontiguousarray(Wv[:, 128 * j:128 * (j + 1)]),
            "wo": np.ascontiguousarray(Wo[512 * j:512 * (j + 1), :]),
            "c2": c2, "s2": s2, "tri": tri,
        })
    return in_maps


def kernel(x, rotary_cos, rotary_sin, Wq, Wk, Wv, Wo, reps=1, phases='ABC', _want_res=False):
    from concourse.bass_utils import run_bass_kernel_spmd
    key = (reps, phases)
    if key not in _CACHE:
        _CACHE[key] = _build(reps, phases)
    nc = _CACHE[key]
    in_maps = _host_prep(x, rotary_cos, rotary_sin, Wq, Wk, Wv, Wo)
    res = run_bass_kernel_spmd(nc, in_maps, list(range(NCORES)))
    out = np.empty((B, S, H), np.float32)
    for b in range(B):
        acc = res.results[4 * b]["out"].astype(np.float64)
        for j in range(1, 4):
            acc += res.results[4 * b + j]["out"]
        out[b] = acc.astype(np.float32)
    if _want_res:
        return out, res
    return out
